# revision 1
# baseline (speedup 1.0000x reference)
"""Trainium2 Bass kernel for the Powderworld BehaviorFluidFlow step.

Contract: kernel(**inputs) takes the FULL unsharded inputs
  world         (16, 20, 512, 512) f32
  rand_movement (16, 1, 512, 512) f32
  rand_interact (16, 1, 512, 512) f32   (unused by the reference)
  rand_element  (16, 1, 512, 512) f32   (unused by the reference)
and returns the FULL (16, 20, 512, 512) f32 output.

Sharding: data-parallel over batch; core k processes batches [2k, 2k+1].
All roll-based neighbor access is along W (axis 3), which stays local.

Layout per (batch, 128-row h-tile): channels are split into group a = the 5
mask channels {0:id, 1:density, 2:gravity, 6:momentum, 8:did-gravity} and
group b = the 15 payload channels {3,4,5,7,9..19}; each group lives in one
SBUF tile (128, nch, 514) with one halo column per side holding the circular
W wrap.  Each pass computes single-channel move masks (a-mask = "pixel takes
the value of its in-direction neighbor", b-mask = shifted a-mask; the two
are disjoint), then blends each group with a plain copy on the Scalar engine
plus two predicated copies on the Vector engine, the int8 mask broadcast
across channels via a step-0 access pattern.  Mask compares run on the
Vector engine at the 2x tensor-scalar rate; the and-chains (0/1 multiplies)
and two payload channels' full blends (exact 0/1-mask arithmetic) run on the
otherwise idle GPSIMD engine, emitted strictly between the mask chains so
they never delay a chain the Vector engine is about to wait on.

The per-iteration stages are software-pipelined by emission order so the
Vector engine always has blend work while GPSIMD finishes a mask chain:
  ... m2(i) b2(i) loadsA(i+2) b1(i+1) loadsB(i+2) fx(i) m1(i+2) ...
The output tile is kept in permuted channel order [a|b]; the store DMAs
scatter the channel groups back to the canonical order.

Element-set membership (id in {0,3,8,9,12,14,15}) is computed exactly in five
Vector-engine tensor-scalar ops via the float exponent trick:
(id+127)<<23 reinterpreted as f32 is exactly 2^id; converting back to int32
gives 1<<id; AND with the set's bitmask 54025 and a nonzero test finish it.
"""
import sys

if '/opt/trn_rl_repo' not in sys.path:
    sys.path.insert(0, '/opt/trn_rl_repo')

import numpy as np
import concourse.bacc as bacc
import concourse.mybir as mybir
import concourse.tile as tile
from concourse.bass_utils import run_bass_kernel_spmd

A = mybir.AluOpType
F32 = mybir.dt.float32
I8 = mybir.dt.int8

B, C, H, W = 16, 20, 512, 512
N_CORES = 8
BPC = B // N_CORES
P = 128

_nc_cache = {}


def build_kernel(bpc=BPC, c=C, h=H, w=W):
    key = (bpc, c, h, w)
    if key in _nc_cache:
        return _nc_cache[key]

    nc = bacc.Bacc("TRN2", target_bir_lowering=False, debug=False,
                   num_devices=N_CORES)
    world = nc.dram_tensor("world", [bpc, c, h, w], F32, kind="ExternalInput")
    rand = nc.dram_tensor("rand", [bpc, h, w], F32, kind="ExternalInput")
    out = nc.dram_tensor("out", [bpc, c, h, w], F32, kind="ExternalOutput")

    WH = w + 2          # haloed width; data in cols [1, w], halos at 0 and w+1
    n_ht = h // P
    MAIN = slice(1, w + 1)
    ca, cb = 5, c - 5   # group sizes (a = mask channels, b = payload)
    NPC = 2             # payload channels blended on GPSIMD instead of DVE
    CBD = cb - NPC      # b-group channels blended with copy_predicated


    # membership set {empty, water, lava, gas, acid, agentK, agentL}
    # = ids {0, 3, 8, 9, 12, 14, 15} = bits of 54025; tested by building
    # 1<<id via the f32 exponent-field trick (exact integer arithmetic).
    MBITS = 54025

    iters = [(b, t) for b in range(bpc) for t in range(n_ht)]
    n = len(iters)
    st = [dict() for _ in range(n)]   # per-iteration tile refs

    with tile.TileContext(nc) as tc:
        with tc.tile_pool(name="ga", bufs=4) as gap, \
             tc.tile_pool(name="gb", bufs=2) as gbp, \
             tc.tile_pool(name="out2", bufs=1) as o2p, \
             tc.tile_pool(name="mk", bufs=9) as mk, \
             tc.tile_pool(name="it", bufs=4) as itp, \
             tc.tile_pool(name="dbl", bufs=2) as dblp, \
             tc.tile_pool(name="amf", bufs=6) as amfp, \
             tc.tile_pool(name="am", bufs=6) as amp, \
             tc.tile_pool(name="pb", bufs=4) as pbp, \
             tc.tile_pool(name="rp", bufs=3) as rp:

            def membership(ch0, out_tile):
                """out_tile = 1.0 where id in bits(MBITS) else 0.0 (all DVE).

                (id+127)<<23 is the f32 bit pattern of 2^id; converting that
                back to int gives 1<<id exactly; AND with MBITS + nonzero.
                """
                IT = itp.tile([P, w], mybir.dt.int32, tag="it")
                VT = itp.tile([P, w], mybir.dt.int32, tag="it")
                nc.vector.tensor_copy(IT[:], ch0)
                nc.vector.tensor_scalar(IT[:], IT[:], 8388608, 1065353216,
                                        A.mult, A.add)
                nc.vector.tensor_copy(VT[:], IT[:].bitcast(F32))
                nc.vector.tensor_scalar(VT[:], VT[:], MBITS, None, A.bitwise_and)
                nc.vector.tensor_scalar(out_tile[:], VT[:], 0, None, A.is_gt)

            def loads_a(i):
                b, t = iters[i]
                hs = slice(t * P, (t + 1) * P)
                s = st[i]
                s['INa'] = gap.tile([P, ca, WH], F32, tag="ga", name=f"INa{i}")
                s['RAND'] = rp.tile([P, w], F32, tag="rand", name=f"RAND{i}")
                T = s['INa']
                nc.sync.dma_start(T[:, 0:3, MAIN],
                                  world[b, 0:3, hs, :].rearrange("c p w -> p c w"))
                nc.sync.dma_start(T[:, 3:4, MAIN],
                                  world[b, 6:7, hs, :].rearrange("c p w -> p c w"))
                nc.sync.dma_start(T[:, 4:5, MAIN],
                                  world[b, 8:9, hs, :].rearrange("c p w -> p c w"))
                nc.sync.dma_start(s['RAND'][:], rand[b, hs, :])
                nc.scalar.copy(T[:, :, 0:1], T[:, :, w:w + 1])
                nc.scalar.copy(T[:, :, w + 1:w + 2], T[:, :, 1:2])

            def loads_b(i):
                b, t = iters[i]
                hs = slice(t * P, (t + 1) * P)
                s = st[i]
                s['INb'] = gbp.tile([P, cb, WH], F32, tag="gb", name=f"INb{i}")
                T = s['INb']
                nc.sync.dma_start(T[:, 0:3, MAIN],
                                  world[b, 3:6, hs, :].rearrange("c p w -> p c w"))
                nc.sync.dma_start(T[:, 3:4, MAIN],
                                  world[b, 7:8, hs, :].rearrange("c p w -> p c w"))
                nc.sync.dma_start(T[:, 4:cb, MAIN],
                                  world[b, 9:c, hs, :].rearrange("c p w -> p c w"))
                nc.scalar.copy(T[:, :, 0:1], T[:, :, w:w + 1])
                nc.scalar.copy(T[:, :, w + 1:w + 2], T[:, :, 1:2])

            def mask_pass(i, which):
                """Move mask (int8, haloed) for a pass; group-a positions:
                0=id, 1=density, 2=gravity, 3=momentum(ch6), 4=didg(ch8).

                which=1: nbr = j-1 (cur at 0:w), overlap-shift = j+1.
                which=2: nbr = j+1 (cur at 2:w+2), overlap-shift = j-1.
                """
                s = st[i]
                cur = s['INa'] if which == 1 else s['O1a']
                nbr = slice(0, w) if which == 1 else slice(2, w + 2)
                RAND = s['RAND']
                FS = mk.tile([P, w], F32, tag="mk")
                AIR = mk.tile([P, w], F32, tag="mk")
                E = mk.tile([P, w], F32, tag="mk")
                NDG = mk.tile([P, w], F32, tag="mk")
                GB = mk.tile([P, w], F32, tag="mk")
                DN = mk.tile([P, w], F32, tag="mk")
                DBL = dblp.tile([P, WH], F32, tag="dbl")
                AMf = amfp.tile([P, WH], F32, tag="amf", name=f"AMf{which}_{i}")
                AM = amp.tile([P, WH], I8, tag="am", name=f"AM{which}_{i}")

                if which == 1:
                    nc.gpsimd.tensor_tensor(FS[:], RAND[:], cur[:, 3, MAIN],
                                            A.add)
                else:
                    # DVE add so the pass-2 chain start never waits on the
                    # (possibly still draining) GPSIMD queue
                    nc.vector.tensor_tensor(FS[:], RAND[:], cur[:, 3, MAIN],
                                            A.add)
                    # + nfm = 2*b1 after pass 1
                    nc.vector.scalar_tensor_tensor(FS[:], s['A1'][:, 2:w + 2],
                                                   2.0, FS[:], A.mult, A.add)
                membership(cur[:, 0, MAIN], E)
                nc.vector.tensor_scalar(AIR[:], cur[:, 0, MAIN], 13.5, None,
                                        A.is_gt)
                nc.vector.scalar_tensor_tensor(NDG[:], cur[:, 4, MAIN], 0.5,
                                               AIR[:], A.is_lt, A.logical_or)
                # gravity is exactly 0/1 so mult == and for the pair test
                nc.gpsimd.tensor_tensor(GB[:], cur[:, 2, MAIN], cur[:, 2, nbr],
                                        A.mult)
                nc.vector.tensor_tensor(DN[:], cur[:, 1, MAIN], cur[:, 1, nbr],
                                        A.is_gt)
                cmp_op = A.is_gt if which == 1 else A.is_le
                nc.vector.scalar_tensor_tensor(FS[:], FS[:], 0.5, DN[:],
                                               cmp_op, A.logical_and)
                nc.gpsimd.tensor_tensor(E[:], E[:], NDG[:], A.mult)
                nc.gpsimd.tensor_tensor(FS[:], FS[:], E[:], A.mult)
                nc.gpsimd.tensor_tensor(DBL[:, MAIN], FS[:], GB[:], A.mult)
                if which == 1:
                    nc.scalar.copy(DBL[:, w + 1:w + 2], DBL[:, 1:2])
                    nc.vector.scalar_tensor_tensor(AMf[:, MAIN], DBL[:, 2:w + 2],
                                                   0.0, DBL[:, MAIN],
                                                   A.is_equal, A.logical_and)
                else:
                    nc.scalar.copy(DBL[:, 0:1], DBL[:, w:w + 1])
                    nc.vector.scalar_tensor_tensor(AMf[:, MAIN], DBL[:, 0:w],
                                                   0.0, DBL[:, MAIN],
                                                   A.is_equal, A.logical_and)
                nc.vector.tensor_copy(AMf[:, 0:1], AMf[:, w:w + 1])
                nc.vector.tensor_copy(AMf[:, w + 1:w + 2], AMf[:, 1:2])
                nc.vector.tensor_copy(AM[:], AMf[:])
                s[f'A{which}f'], s[f'A{which}'] = AMf, AM

            def pool_blend(s, which, cur, curch, outv):
                """Exact one-channel blend on GPSIMD:
                out = cur*(1-a-b) + a*nbr + b*opp (masks exactly 0/1)."""
                AMf = s[f'A{which}f']
                nbr = slice(0, w) if which == 1 else slice(2, w + 2)
                opp = slice(2, w + 2) if which == 1 else slice(0, w)
                NM = s[f'NM{which}']
                X = pbp.tile([P, w], F32, tag="pb", name=f"X{which}")
                nc.gpsimd.tensor_tensor(X[:], cur[:, curch, MAIN], NM[:], A.mult)
                nc.gpsimd.tensor_tensor(outv, cur[:, curch, nbr], AMf[:, MAIN],
                                        A.mult)
                nc.gpsimd.tensor_tensor(outv, outv, X[:], A.add)
                nc.gpsimd.tensor_tensor(X[:], cur[:, curch, opp], AMf[:, opp],
                                        A.mult)
                nc.gpsimd.tensor_tensor(outv, outv, X[:], A.add)

            def make_nm(s, which):
                # NM = 1 - a - b (exactly 0 where the pixel moves, else 1)
                AMf = s[f'A{which}f']
                opp = slice(2, w + 2) if which == 1 else slice(0, w)
                NM = pbp.tile([P, w], F32, tag="pb", name=f"NM{which}")
                nc.gpsimd.tensor_tensor(NM[:], AMf[:, MAIN], AMf[:, opp], A.add)
                nc.gpsimd.tensor_scalar(NM[:], NM[:], -1.0, None, A.mult)
                nc.gpsimd.tensor_scalar(NM[:], NM[:], 1.0, None, A.add)
                s[f'NM{which}'] = NM

            def blend1_dve(i):
                s = st[i]
                A1 = s['A1']
                s['O1a'] = gap.tile([P, ca, WH], F32, tag="ga", name=f"O1a{i}")
                s['O1b'] = gbp.tile([P, cb, WH], F32, tag="gb", name=f"O1b{i}")
                for IN, O1, nch in ((s['INa'], s['O1a'], ca),
                                    (s['INb'], s['O1b'], CBD)):
                    am = A1[:, MAIN].unsqueeze(1).broadcast_to((P, nch, w))
                    bm = A1[:, 2:w + 2].unsqueeze(1).broadcast_to((P, nch, w))
                    nc.scalar.copy(O1[:, 0:nch, MAIN], IN[:, 0:nch, MAIN])
                    nc.vector.copy_predicated(O1[:, 0:nch, MAIN], am,
                                              IN[:, 0:nch, 0:w])
                    nc.vector.copy_predicated(O1[:, 0:nch, MAIN], bm,
                                              IN[:, 0:nch, 2:w + 2])
                nc.scalar.copy(s['O1a'][:, :, 0:1], s['O1a'][:, :, w:w + 1])
                nc.scalar.copy(s['O1a'][:, :, w + 1:w + 2], s['O1a'][:, :, 1:2])
                O1b = s['O1b']
                nc.scalar.copy(O1b[:, 0:CBD, 0:1], O1b[:, 0:CBD, w:w + 1])
                nc.scalar.copy(O1b[:, 0:CBD, w + 1:w + 2], O1b[:, 0:CBD, 1:2])

            def blend1_pool(i):
                s = st[i]
                make_nm(s, 1)
                for k in range(CBD, cb):
                    pool_blend(s, 1, s['INb'], k, s['O1b'][:, k, MAIN])
                O1b = s['O1b']
                nc.scalar.copy(O1b[:, CBD:cb, 0:1], O1b[:, CBD:cb, w:w + 1])
                nc.scalar.copy(O1b[:, CBD:cb, w + 1:w + 2], O1b[:, CBD:cb, 1:2])

            def blend2(i):
                b, t = iters[i]
                hs = slice(t * P, (t + 1) * P)
                s = st[i]
                A2 = s['A2']
                s['O2'] = o2p.tile([P, c, WH], F32, tag="out2", name=f"O2_{i}")
                O2 = s['O2']
                for O1, o2sl, nch in ((s['O1a'], slice(0, ca), ca),
                                      (s['O1b'], slice(ca, ca + CBD), CBD)):
                    am = A2[:, MAIN].unsqueeze(1).broadcast_to((P, nch, w))
                    bm = A2[:, 0:w].unsqueeze(1).broadcast_to((P, nch, w))
                    nc.scalar.copy(O2[:, o2sl, MAIN], O1[:, 0:nch, MAIN])
                    nc.vector.copy_predicated(O2[:, o2sl, MAIN], am,
                                              O1[:, 0:nch, 2:w + 2])
                    nc.vector.copy_predicated(O2[:, o2sl, MAIN], bm,
                                              O1[:, 0:nch, 0:w])
                # store all channels except 6 (position 3; fixed up in fx).
                # O2 channel order is [0,1,2,6,8 | 3,4,5,7 | 9..19].
                nc.scalar.dma_start(out[b, 0:3, hs, :].rearrange("c p w -> p c w"),
                                    O2[:, 0:3, MAIN])
                nc.scalar.dma_start(out[b, 8:9, hs, :].rearrange("c p w -> p c w"),
                                    O2[:, 4:5, MAIN])
                nc.scalar.dma_start(out[b, 3:6, hs, :].rearrange("c p w -> p c w"),
                                    O2[:, 5:8, MAIN])
                nc.scalar.dma_start(out[b, 7:8, hs, :].rearrange("c p w -> p c w"),
                                    O2[:, 8:9, MAIN])

            def blend2_pool(i):
                b, t = iters[i]
                hs = slice(t * P, (t + 1) * P)
                s = st[i]
                O2 = s['O2']
                make_nm(s, 2)
                for k in range(CBD, cb):
                    pool_blend(s, 2, s['O1b'], k, O2[:, ca + k, MAIN])
                nc.scalar.dma_start(out[b, 9:c, hs, :].rearrange("c p w -> p c w"),
                                    O2[:, 9:c, MAIN])

            def fixup(i):
                b, t = iters[i]
                hs = slice(t * P, (t + 1) * P)
                s = st[i]
                O2 = s['O2']
                NF = mk.tile([P, w], F32, tag="mk")
                FLI = amp.tile([P, w], I8, tag="am", name=f"FLI{i}")
                # nf = 2*b1 - 2*b2 (f32 masks, exact small integers)
                nc.gpsimd.tensor_tensor(NF[:], s['A1f'][:, 2:w + 2],
                                        s['A2f'][:, 0:w], A.subtract)
                nc.gpsimd.tensor_scalar(NF[:], NF[:], 2.0, None, A.mult)
                membership(O2[:, 0, MAIN], FLI)
                nc.vector.copy_predicated(O2[:, 3, MAIN], FLI[:], NF[:])
                nc.scalar.dma_start(out[b, 6, hs, :], O2[:, 3, MAIN])

            # ---- software-pipelined emission -------------------------------
            loads_a(0)
            loads_b(0)
            mask_pass(0, 1)
            if n > 1:
                loads_a(1)
            blend1_dve(0)
            if n > 1:
                loads_b(1)
                mask_pass(1, 1)
            for i in range(n):
                mask_pass(i, 2)
                blend1_pool(i)
                blend2(i)
                blend2_pool(i)
                if i + 2 < n:
                    loads_a(i + 2)
                if i + 1 < n:
                    blend1_dve(i + 1)
                if i + 2 < n:
                    loads_b(i + 2)
                fixup(i)
                if i + 2 < n:
                    mask_pass(i + 2, 1)

    nc.compile()
    _nc_cache[key] = nc
    return nc


def kernel(world, rand_movement, rand_interact, rand_element):
    del rand_interact, rand_element
    nc = build_kernel()
    in_maps = []
    for k in range(N_CORES):
        bs = slice(k * BPC, (k + 1) * BPC)
        in_maps.append({
            "world": np.ascontiguousarray(world[bs]),
            "rand": np.ascontiguousarray(rand_movement[bs, 0]),
        })
    res = run_bass_kernel_spmd(nc, in_maps, list(range(N_CORES)))
    return np.concatenate([res.results[k]["out"] for k in range(N_CORES)], axis=0)



# revision 6
# speedup vs baseline: 1.3805x; 1.3805x over previous
"""Trainium2 Bass kernel for the Powderworld BehaviorFluidFlow step.

Contract: kernel(**inputs) takes the FULL unsharded inputs
  world         (16, 20, 512, 512) f32
  rand_movement (16, 1, 512, 512) f32
  rand_interact (16, 1, 512, 512) f32   (unused by the reference)
  rand_element  (16, 1, 512, 512) f32   (unused by the reference)
and returns the FULL (16, 20, 512, 512) f32 output.

Sharding: data-parallel over batch; core k processes batches [2k, 2k+1].
All roll-based neighbor access is along W (axis 3), which stays local.

Architecture (v2): per (batch, 128-row h-tile) the 20 channels live in one
haloed SBUF tile Told (128, 20, 514), tile channel order
[id, dens, grav, mom, didg | payload...].  Each pass computes a single
a-mask ("pixel j takes its in-direction neighbor"); the b-mask is the
a-mask at a shifted column (the move is a pairwise swap, so b = shift(a)).
The blend ping-pongs DVE-assigned channels Told->Tnew->Told with one
batched Activation base-copy + two batched copy_predicated per pass
(CopyPredicated has no DVE fast mode but this is still the cheapest
per-element select on TRN2); the Pool engine moves its channels in place
with an exact XOR pair-swap (f = a32 & (x ^ x_nbr); x ^= f ^ f_shift),
which needs no base copy.  Mask-chain binary merges run on Pool
(logical ops at 0.6 efficiency), the int-trick element-membership and
mask finishers on DVE, so DVE ~= Pool ~= 43us/iter with DMA ~30us/iter
underneath.
"""
import sys

if '/opt/trn_rl_repo' not in sys.path:
    sys.path.insert(0, '/opt/trn_rl_repo')

import numpy as np
import concourse.bacc as bacc
import concourse.mybir as mybir
import concourse.tile as tile
from concourse.bass_utils import run_bass_kernel_spmd

A = mybir.AluOpType
F32 = mybir.dt.float32
I8 = mybir.dt.int8
I32 = mybir.dt.int32

B, C, H, W = 16, 20, 512, 512
N_CORES = 8
BPC = B // N_CORES
P = 128

_nc_cache = {}

# tile channel order: positions 0..4 = the mask channels, then payload
TILE_CH = [0, 1, 2, 6, 8, 3, 4, 5, 7, 9, 10, 11, 12, 13, 14, 15, 16, 17, 18, 19]
# contiguous runs (dram_start, dram_stop, tile_start) for load/store DMAs
RUNS = [(0, 3, 0), (6, 7, 3), (8, 9, 4), (3, 6, 5), (7, 8, 8), (9, 20, 9)]

# membership set {empty, water, lava, gas, acid, agentK, agentL}
# = ids {0, 3, 8, 9, 12, 14, 15} = bits of 54025
MBITS = 54025

ND = 15        # number of DVE-blended channels (must include the 5 mask chs)
NPOOL = C - ND  # pool XOR-blended channels (payload only)


def build_kernel(bpc=BPC, c=C, h=H, w=W):
    key = (bpc, c, h, w)
    if key in _nc_cache:
        return _nc_cache[key]

    nc = bacc.Bacc("TRN2", target_bir_lowering=False, debug=False,
                   num_devices=N_CORES)
    world = nc.dram_tensor("world", [bpc, c, h, w], F32, kind="ExternalInput")
    rand = nc.dram_tensor("rand", [bpc, h, w], F32, kind="ExternalInput")
    out = nc.dram_tensor("out", [bpc, c, h, w], F32, kind="ExternalOutput")

    WH = w + 2          # haloed width; data in cols [1, w], halos at 0, w+1
    n_ht = h // P
    MAIN = slice(1, w + 1)
    LEFT = slice(0, w)
    RIGHT = slice(2, w + 2)

    iters = [(b, t) for b in range(bpc) for t in range(n_ht)]
    n = len(iters)
    st = [dict() for _ in range(n)]

    with tile.TileContext(nc) as tc:
        with tc.tile_pool(name="told", bufs=2) as toldp, \
             tc.tile_pool(name="tnew", bufs=1) as tnewp, \
             tc.tile_pool(name="rp", bufs=2) as rp, \
             tc.tile_pool(name="am", bufs=4) as amp, \
             tc.tile_pool(name="mk", bufs=7) as mk, \
             tc.tile_pool(name="it", bufs=4) as itp, \
             tc.tile_pool(name="px", bufs=2) as pxp:

            def membership(ch0, out_tile):
                """out = 1 where id in bits(MBITS) else 0 (exact int trick).

                (id+127)<<23 is the f32 bit pattern of 2^id; converting back
                to int gives 1<<id; AND with MBITS + nonzero test.
                """
                IT = itp.tile([P, w], I32, tag="it", name="IT")
                VT = itp.tile([P, w], I32, tag="it", name="VT")
                nc.vector.tensor_copy(IT[:], ch0)
                nc.vector.tensor_scalar(IT[:], IT[:], 8388608, 1065353216,
                                        A.mult, A.add)
                nc.vector.tensor_copy(VT[:], IT[:].bitcast(F32))
                nc.vector.tensor_scalar(out_tile[:], VT[:], MBITS, 0,
                                        A.bitwise_and, A.is_gt)

            def loads(i):
                b, t = iters[i]
                hs = slice(t * P, (t + 1) * P)
                s = st[i]
                s['T'] = toldp.tile([P, c, WH], F32, tag="told", name=f"T{i}")
                s['RAND'] = rp.tile([P, w], F32, tag="rand", name=f"RAND{i}")
                T = s['T']
                for d0, d1, t0 in RUNS:
                    nc.sync.dma_start(
                        T[:, t0:t0 + (d1 - d0), MAIN],
                        world[b, d0:d1, hs, :].rearrange("c p w -> p c w"))
                nc.sync.dma_start(s['RAND'][:], rand[b, hs, :])
                nc.scalar.copy(T[:, :, 0:1], T[:, :, w:w + 1])
                nc.scalar.copy(T[:, :, w + 1:w + 2], T[:, :, 1:2])

            def mask_pass(i, p2):
                """a-mask for a pass.  p2=False: fall-left pass (neighbor is
                j-1, overlap shift +1); p2=True: fall-right (neighbor j+1,
                overlap shift -1).  Reads the mask channels of tile `cur`
                (Told for pass 1, Tnew for pass 2)."""
                s = st[i]
                cur = s['T'] if not p2 else s['TN']
                nbr = LEFT if not p2 else RIGHT
                RAND = s['RAND']
                cid = cur[:, 0, MAIN]
                FS = mk.tile([P, w], F32, tag="mk", name="FS")
                DN = mk.tile([P, w], F32, tag="mk", name="DN")
                E = mk.tile([P, w], F32, tag="mk", name="E")
                M4 = mk.tile([P, w], F32, tag="mk", name="M4")
                M3 = mk.tile([P, w], F32, tag="mk", name="M3")
                GB = mk.tile([P, w], F32, tag="mk", name="GB")
                LH = mk.tile([P, WH], F32, tag="mkh", name="LH")
                AMf = amp.tile([P, WH], F32, tag="amf", name=f"AMf{p2}_{i}")
                AM8 = amp.tile([P, WH], I8, tag="am8", name=f"AM8{p2}_{i}")

                # --- Pool: the two-input merges -------------------------
                if not p2:
                    nc.gpsimd.tensor_tensor(FS[:], RAND[:], cur[:, 3, MAIN],
                                            A.add)
                else:
                    nc.gpsimd.tensor_tensor(FS[:], RAND[:], cur[:, 3, MAIN],
                                            A.add)
                    # + nfm = 2*b1 = 2*a1(j+1)
                    nc.gpsimd.scalar_tensor_tensor(
                        FS[:], s['A1f'][:, RIGHT], 2.0, FS[:], A.mult, A.add)
                nc.gpsimd.tensor_tensor(DN[:], cur[:, 1, MAIN], cur[:, 1, nbr],
                                        A.is_gt)
                cmp_op = A.is_gt if not p2 else A.is_le
                nc.gpsimd.scalar_tensor_tensor(M3[:], FS[:], 0.5, DN[:],
                                               cmp_op, A.logical_and)
                nc.gpsimd.tensor_tensor(GB[:], cur[:, 2, MAIN], cur[:, 2, nbr],
                                        A.logical_and)
                # --- DVE: membership (int trick) ------------------------
                membership(cid, E)
                # --- Pool: remaining merges -----------------------------
                # M4 = E & (didg < 0.5);  M4 |= (id > 13.5)  [air override]
                nc.gpsimd.scalar_tensor_tensor(M4[:], cur[:, 4, MAIN], 0.5,
                                               E[:], A.is_lt, A.logical_and)
                nc.gpsimd.scalar_tensor_tensor(M4[:], cid, 13.5, M4[:],
                                               A.is_gt, A.logical_or)
                nc.gpsimd.tensor_tensor(M3[:], M3[:], M4[:], A.logical_and)
                nc.gpsimd.tensor_tensor(LH[:, MAIN], M3[:], GB[:],
                                        A.logical_and)
                # --- halo of L, then overlap removal on DVE -------------
                if not p2:
                    nc.scalar.copy(LH[:, w + 1:w + 2], LH[:, 1:2])
                    nc.vector.scalar_tensor_tensor(AMf[:, MAIN], LH[:, RIGHT],
                                                   0.0, LH[:, MAIN],
                                                   A.is_equal, A.logical_and)
                    nc.scalar.copy(AMf[:, w + 1:w + 2], AMf[:, 1:2])
                    nc.vector.tensor_copy(AM8[:, 1:w + 2], AMf[:, 1:w + 2])
                else:
                    nc.scalar.copy(LH[:, 0:1], LH[:, w:w + 1])
                    nc.vector.scalar_tensor_tensor(AMf[:, MAIN], LH[:, LEFT],
                                                   0.0, LH[:, MAIN],
                                                   A.is_equal, A.logical_and)
                    nc.scalar.copy(AMf[:, 0:1], AMf[:, w:w + 1])
                    nc.vector.tensor_copy(AM8[:, 0:w + 1], AMf[:, 0:w + 1])
                which = 2 if p2 else 1
                s[f'A{which}f'], s[f'A{which}8'] = AMf, AM8
                # int32 all-ones mask for the pool XOR channels
                A32 = amp.tile([P, WH], I32, tag="a32", name=f"A32{which}_{i}")
                if not p2:
                    nc.vector.tensor_scalar(A32[:, 1:w + 2], AMf[:, 1:w + 2],
                                            -1.0, None, A.mult)
                else:
                    nc.vector.tensor_scalar(A32[:, 0:w + 1], AMf[:, 0:w + 1],
                                            -1.0, None, A.mult)
                s[f'A32_{which}'] = A32

            def blend_dve(i, p2):
                """Ping-pong CP blend of the ND dve channels.
                pass1: Tnew <- Told;  pass2: Told <- Tnew."""
                s = st[i]
                if not p2:
                    s['TN'] = tnewp.tile([P, ND, WH], F32, tag="tnew",
                                         name=f"TN{i}")
                    src, dst = s['T'], s['TN']
                    AM8 = s['A18']
                    am = AM8[:, MAIN]      # a: take in-dir nbr (j-1)
                    bm = AM8[:, RIGHT]     # b[j] = a[j+1]
                    asrc, bsrc = LEFT, RIGHT
                else:
                    src, dst = s['TN'], s['T']
                    AM8 = s['A28']
                    am = AM8[:, MAIN]      # a: take nbr j+1
                    bm = AM8[:, LEFT]      # b[j] = a[j-1]
                    asrc, bsrc = RIGHT, LEFT
                # base copy on Act: two chunks so mask channels land first
                # and unblock the next mask chain / CP stage sooner.
                for c0, c1 in ((0, 5), (5, ND)):
                    nch = c1 - c0
                    nc.scalar.copy(dst[:, c0:c1, :], src[:, c0:c1, :])
                    amb = am.unsqueeze(1).broadcast_to((P, nch, w))
                    bmb = bm.unsqueeze(1).broadcast_to((P, nch, w))
                    nc.vector.copy_predicated(dst[:, c0:c1, MAIN], amb,
                                              src[:, c0:c1, asrc])
                    nc.vector.copy_predicated(dst[:, c0:c1, MAIN], bmb,
                                              src[:, c0:c1, bsrc])
                if not p2:
                    # refresh halo columns for the pass-2 stages
                    nc.scalar.copy(dst[:, :, 0:1], dst[:, :, w:w + 1])
                    nc.scalar.copy(dst[:, :, w + 1:w + 2], dst[:, :, 1:2])

            def blend_pool(i, p2):
                """Exact in-place XOR pair-swap of the NPOOL tail channels of
                Told.  pass1 pairs (j-1, j) via a1[j]; pass2 pairs (j, j+1)
                via a2[j]."""
                s = st[i]
                T = s['T']
                which = 2 if p2 else 1
                A32 = s[f'A32_{which}']
                ti = T[:, ND:c, :].bitcast(I32)
                X = pxp.tile([P, NPOOL, WH], I32, tag="px", name=f"X{which}")
                if not p2:
                    # f[j] = a1[j] & (x[j] ^ x[j-1]), j in [1..w+1]
                    fs = slice(1, w + 2)
                    fl = slice(0, w + 1)
                    a32 = A32[:, fs].unsqueeze(1).broadcast_to(
                        (P, NPOOL, w + 1))
                    nc.gpsimd.tensor_tensor(X[:, :, fs], ti[:, :, fs],
                                            ti[:, :, fl], A.bitwise_xor)
                    nc.gpsimd.tensor_tensor(X[:, :, fs], X[:, :, fs], a32,
                                            A.bitwise_and)
                    # x ^= f[j] ^ f[j+1]  (two aligned in-place xors)
                    osl = RIGHT
                else:
                    # f[j] = a2[j] & (x[j] ^ x[j+1]), j in [0..w]
                    fs = slice(0, w + 1)
                    fr = slice(1, w + 2)
                    a32 = A32[:, fs].unsqueeze(1).broadcast_to(
                        (P, NPOOL, w + 1))
                    nc.gpsimd.tensor_tensor(X[:, :, fs], ti[:, :, fs],
                                            ti[:, :, fr], A.bitwise_xor)
                    nc.gpsimd.tensor_tensor(X[:, :, fs], X[:, :, fs], a32,
                                            A.bitwise_and)
                    # x ^= f[j] ^ f[j-1]
                    osl = LEFT
                nc.gpsimd.tensor_tensor(ti[:, :, MAIN], ti[:, :, MAIN],
                                        X[:, :, MAIN], A.bitwise_xor)
                nc.gpsimd.tensor_tensor(ti[:, :, MAIN], ti[:, :, MAIN],
                                        X[:, :, osl], A.bitwise_xor)
                if not p2:
                    # refresh halos of the pool channels for pass 2
                    nc.scalar.copy(T[:, ND:c, 0:1], T[:, ND:c, w:w + 1])
                    nc.scalar.copy(T[:, ND:c, w + 1:w + 2], T[:, ND:c, 1:2])

            def fixup_store(i):
                b, t = iters[i]
                hs = slice(t * P, (t + 1) * P)
                s = st[i]
                T = s['T']
                NF = mk.tile([P, w], F32, tag="mk", name="NF")
                FLI = mk.tile([P, w], F32, tag="mk", name="FLI")
                FLI8 = amp.tile([P, w], I8, tag="fli8", name=f"FLI8_{i}")
                # nfm = 2*b1 - 2*b2 = 2*(a1[j+1] - a2[j-1])
                nc.vector.tensor_tensor(NF[:], s['A1f'][:, RIGHT],
                                        s['A2f'][:, LEFT], A.subtract)
                nc.vector.tensor_scalar(NF[:], NF[:], 2.0, None, A.mult)
                membership(T[:, 0, MAIN], FLI)
                nc.vector.tensor_copy(FLI8[:], FLI[:])
                nc.vector.copy_predicated(T[:, 3, MAIN], FLI8[:], NF[:])
                # store: tile order back to canonical dram channels
                for d0, d1, t0 in RUNS:
                    q = nc.scalar if d0 != 6 else nc.scalar
                    q.dma_start(
                        out[b, d0:d1, hs, :].rearrange("c p w -> p c w"),
                        T[:, t0:t0 + (d1 - d0), MAIN])

            # ---- software-pipelined emission -------------------------------
            loads(0)
            for i in range(n):
                mask_pass(i, False)
                blend_pool(i, False)
                blend_dve(i, False)
                if i + 1 < n:
                    loads(i + 1)
                mask_pass(i, True)
                blend_pool(i, True)
                blend_dve(i, True)
                fixup_store(i)

    nc.compile()
    _nc_cache[key] = nc
    return nc


def kernel(world, rand_movement, rand_interact, rand_element):
    del rand_interact, rand_element
    nc = build_kernel()
    perm = np.asarray(TILE_CH)
    in_maps = []
    for k in range(N_CORES):
        bs = slice(k * BPC, (k + 1) * BPC)
        in_maps.append({
            "world": np.ascontiguousarray(world[bs]),
            "rand": np.ascontiguousarray(rand_movement[bs, 0]),
        })
    res = run_bass_kernel_spmd(nc, in_maps, list(range(N_CORES)))
    return np.concatenate([res.results[k]["out"] for k in range(N_CORES)], axis=0)


# revision 7
# speedup vs baseline: 1.3899x; 1.0068x over previous
"""Trainium2 Bass kernel for the Powderworld BehaviorFluidFlow step.

Contract: kernel(**inputs) takes the FULL unsharded inputs
  world         (16, 20, 512, 512) f32
  rand_movement (16, 1, 512, 512) f32
  rand_interact (16, 1, 512, 512) f32   (unused by the reference)
  rand_element  (16, 1, 512, 512) f32   (unused by the reference)
and returns the FULL (16, 20, 512, 512) f32 output.

Sharding: data-parallel over batch; core k processes batches [2k, 2k+1].
All roll-based neighbor access is along W (axis 3), which stays local.

Architecture: per (batch, 128-row h-tile) the channels live in two haloed
SBUF tiles: TD (128, 15, 514) = [id, dens, grav, mom, didg | 10 payload]
blended on DVE, TP (128, 5, 514) = 5 payload channels blended on Pool.
Each pass computes one a-mask ("pixel j takes its in-direction neighbor");
the b-mask is the a-mask at a shifted column (the move is a pairwise swap,
so b = shift(a)).  DVE channels ping-pong TD->TN->TD with one batched
Activation base-copy (hoisted so it overlaps the mask chain) + two batched
copy_predicated per pass.  Pool channels move in place with an exact XOR
pair-swap (f = a32 & (x ^ x_nbr); x ^= f; x ^= f_shift) -- no base copy.
Mask-chain two-input merges run on Pool (logical ALU at 0.6 efficiency),
the int-trick element-membership and mask finishers on DVE; halo refreshes
are split so the pass-2 chain unblocks after the 5 mask channels' CPs.
"""
import sys

if '/opt/trn_rl_repo' not in sys.path:
    sys.path.insert(0, '/opt/trn_rl_repo')

import numpy as np
import concourse.bacc as bacc
import concourse.mybir as mybir
import concourse.tile as tile
from concourse.bass_utils import run_bass_kernel_spmd

A = mybir.AluOpType
F32 = mybir.dt.float32
I8 = mybir.dt.int8
I32 = mybir.dt.int32

B, C, H, W = 16, 20, 512, 512
N_CORES = 8
BPC = B // N_CORES
P = 128

_nc_cache = {}

ND = 15        # DVE-blended channels (the 5 mask channels + 10 payload)
NPOOL = C - ND  # pool XOR-blended channels (payload only)

# TD channel order -> dram channels [0,1,2,6,8,3,4,5,7,9,10,11,12,13,14]
# TP channel order -> dram channels [15,16,17,18,19]
TD_RUNS = [(0, 3, 0), (6, 7, 3), (8, 9, 4), (3, 6, 5), (7, 8, 8), (9, 15, 9)]
TP_RUNS = [(15, 20, 0)]

# membership set {empty, water, lava, gas, acid, agentK, agentL}
# = ids {0, 3, 8, 9, 12, 14, 15} = bits of 54025
MBITS = 54025


def build_kernel(bpc=BPC, c=C, h=H, w=W):
    key = (bpc, c, h, w)
    if key in _nc_cache:
        return _nc_cache[key]

    nc = bacc.Bacc("TRN2", target_bir_lowering=False, debug=False,
                   num_devices=N_CORES)
    world = nc.dram_tensor("world", [bpc, c, h, w], F32, kind="ExternalInput")
    rand = nc.dram_tensor("rand", [bpc, h, w], F32, kind="ExternalInput")
    out = nc.dram_tensor("out", [bpc, c, h, w], F32, kind="ExternalOutput")

    WH = w + 2          # haloed width; data in cols [1, w], halos at 0, w+1
    n_ht = h // P
    MAIN = slice(1, w + 1)
    LEFT = slice(0, w)
    RIGHT = slice(2, w + 2)

    iters = [(b, t) for b in range(bpc) for t in range(n_ht)]
    n = len(iters)
    st = [dict() for _ in range(n)]

    with tile.TileContext(nc) as tc:
        with tc.tile_pool(name="td", bufs=2) as tdp, \
             tc.tile_pool(name="tp", bufs=2) as tpp, \
             tc.tile_pool(name="tn", bufs=2) as tnp, \
             tc.tile_pool(name="rp", bufs=2) as rp, \
             tc.tile_pool(name="am", bufs=3) as amp, \
             tc.tile_pool(name="mk", bufs=6) as mk, \
             tc.tile_pool(name="mh", bufs=3) as mh, \
             tc.tile_pool(name="it", bufs=2) as itp, \
             tc.tile_pool(name="px", bufs=2) as pxp:

            def membership(ch0, out_tile):
                """out = 1 where id in bits(MBITS) else 0 (exact int trick).

                (id+127)<<23 is the f32 bit pattern of 2^id; converting back
                to int gives 1<<id; AND with MBITS + nonzero test.
                """
                IT = itp.tile([P, w], I32, tag="it", name="IT")
                VT = itp.tile([P, w], I32, tag="it", name="VT")
                nc.vector.tensor_copy(IT[:], ch0)
                nc.vector.tensor_scalar(IT[:], IT[:], 8388608, 1065353216,
                                        A.mult, A.add)
                nc.vector.tensor_copy(VT[:], IT[:].bitcast(F32))
                nc.vector.tensor_scalar(out_tile[:], VT[:], MBITS, 0,
                                        A.bitwise_and, A.is_gt)

            def loads(i):
                b, t = iters[i]
                hs = slice(t * P, (t + 1) * P)
                s = st[i]
                s['TD'] = tdp.tile([P, ND, WH], F32, tag="td", name=f"TD{i}")
                s['TP'] = tpp.tile([P, NPOOL, WH], F32, tag="tp", name=f"TP{i}")
                s['RAND'] = rp.tile([P, w], F32, tag="rand", name=f"RAND{i}")
                TD, TP = s['TD'], s['TP']
                for d0, d1, t0 in TD_RUNS:
                    nc.sync.dma_start(
                        TD[:, t0:t0 + (d1 - d0), MAIN],
                        world[b, d0:d1, hs, :].rearrange("c p w -> p c w"))
                for d0, d1, t0 in TP_RUNS:
                    nc.sync.dma_start(
                        TP[:, t0:t0 + (d1 - d0), MAIN],
                        world[b, d0:d1, hs, :].rearrange("c p w -> p c w"))
                nc.sync.dma_start(s['RAND'][:], rand[b, hs, :])
                nc.scalar.copy(TD[:, :, 0:1], TD[:, :, w:w + 1])
                nc.scalar.copy(TD[:, :, w + 1:w + 2], TD[:, :, 1:2])
                nc.scalar.copy(TP[:, :, 0:1], TP[:, :, w:w + 1])
                nc.scalar.copy(TP[:, :, w + 1:w + 2], TP[:, :, 1:2])

            def base1(i):
                # Act: TN <- TD (both chunks); no mask dependence, so this
                # overlaps the pass-1 mask chain entirely.
                s = st[i]
                s['TN'] = tnp.tile([P, ND, WH], F32, tag="tn", name=f"TN{i}")
                nc.scalar.copy(s['TN'][:, 0:5, :], s['TD'][:, 0:5, :])
                nc.scalar.copy(s['TN'][:, 5:ND, :], s['TD'][:, 5:ND, :])

            def base2(i):
                # Act: TD <- TN; chunk (0,5) only needs the pass-1 CPs of
                # the mask channels, so emit as two chunks again.
                s = st[i]
                nc.scalar.copy(s['TD'][:, 0:5, :], s['TN'][:, 0:5, :])
                nc.scalar.copy(s['TD'][:, 5:ND, :], s['TN'][:, 5:ND, :])

            def mask_pass(i, p2):
                """a-mask for a pass.  p2=False: fall-left pass (neighbor is
                j-1, overlap shift +1); p2=True: fall-right (neighbor j+1,
                overlap shift -1).  Mask channels come from TD (pass 1) or
                TN (pass 2)."""
                s = st[i]
                cur = s['TD'] if not p2 else s['TN']
                nbr = LEFT if not p2 else RIGHT
                RAND = s['RAND']
                cid = cur[:, 0, MAIN]
                FS = mk.tile([P, w], F32, tag="mk", name="FS")
                DN = mk.tile([P, w], F32, tag="mk", name="DN")
                E = mk.tile([P, w], F32, tag="mk", name="E")
                M4 = mk.tile([P, w], F32, tag="mk", name="M4")
                GB = mk.tile([P, w], F32, tag="mk", name="GB")
                LH = mh.tile([P, WH], F32, tag="mkh", name="LH")
                AM8 = amp.tile([P, WH], I8, tag="am8", name=f"AM8{p2}_{i}")
                AMf = amp.tile([P, WH], F32, tag="amf", name=f"AMf{p2}_{i}")

                # --- DVE: membership (int trick), ready early ------------
                membership(cid, E)
                # --- Pool: the two-input merges --------------------------
                nc.gpsimd.tensor_tensor(FS[:], RAND[:], cur[:, 3, MAIN],
                                        A.add)
                if p2:
                    # + nfm = 2*b1 = 2*a1(j+1)
                    nc.gpsimd.scalar_tensor_tensor(
                        FS[:], s['A1f'][:, RIGHT], 2.0, FS[:], A.mult, A.add)
                nc.gpsimd.tensor_tensor(DN[:], cur[:, 1, MAIN], cur[:, 1, nbr],
                                        A.is_gt)
                cmp_op = A.is_gt if not p2 else A.is_le
                nc.gpsimd.scalar_tensor_tensor(DN[:], FS[:], 0.5, DN[:],
                                               cmp_op, A.logical_and)
                nc.gpsimd.tensor_tensor(GB[:], cur[:, 2, MAIN], cur[:, 2, nbr],
                                        A.logical_and)
                # M4 = (E & didg<0.5) | id>13.5   [air override]
                nc.gpsimd.scalar_tensor_tensor(M4[:], cur[:, 4, MAIN], 0.5,
                                               E[:], A.is_lt, A.logical_and)
                nc.gpsimd.scalar_tensor_tensor(M4[:], cid, 13.5, M4[:],
                                               A.is_gt, A.logical_or)
                nc.gpsimd.tensor_tensor(DN[:], DN[:], M4[:], A.logical_and)
                nc.gpsimd.tensor_tensor(LH[:, MAIN], DN[:], GB[:],
                                        A.logical_and)
                # --- halo of L, overlap removal, masks on DVE ------------
                which = 2 if p2 else 1
                if not p2:
                    nc.scalar.copy(LH[:, w + 1:w + 2], LH[:, 1:2])
                    nc.vector.scalar_tensor_tensor(AMf[:, MAIN], LH[:, RIGHT],
                                                   0.0, LH[:, MAIN],
                                                   A.is_equal, A.logical_and)
                    nc.scalar.copy(AMf[:, w + 1:w + 2], AMf[:, 1:2])
                    vs = slice(1, w + 2)
                else:
                    nc.scalar.copy(LH[:, 0:1], LH[:, w:w + 1])
                    nc.vector.scalar_tensor_tensor(AMf[:, MAIN], LH[:, LEFT],
                                                   0.0, LH[:, MAIN],
                                                   A.is_equal, A.logical_and)
                    nc.scalar.copy(AMf[:, 0:1], AMf[:, w:w + 1])
                    vs = slice(0, w + 1)
                nc.vector.tensor_copy(AM8[:, vs], AMf[:, vs])
                s[f'A{which}f'], s[f'A{which}8'] = AMf, AM8
                # int32 all-ones mask for the pool XOR channels
                A32 = amp.tile([P, WH], I32, tag="a32", name=f"A32{which}_{i}")
                nc.vector.tensor_scalar(A32[:, vs], AMf[:, vs], -1.0, None,
                                        A.mult)
                s[f'A32_{which}'] = A32

            def blend_cps(i, p2):
                """The two batched copy_predicated per chunk (DVE), plus the
                halo refreshes (Act) split by chunk."""
                s = st[i]
                if not p2:
                    src, dst = s['TD'], s['TN']
                    AM8 = s['A18']
                    am, bm = AM8[:, MAIN], AM8[:, RIGHT]   # b[j] = a[j+1]
                    asrc, bsrc = LEFT, RIGHT
                else:
                    src, dst = s['TN'], s['TD']
                    AM8 = s['A28']
                    am, bm = AM8[:, MAIN], AM8[:, LEFT]    # b[j] = a[j-1]
                    asrc, bsrc = RIGHT, LEFT
                for c0, c1 in ((0, 5), (5, ND)):
                    nch = c1 - c0
                    amb = am.unsqueeze(1).broadcast_to((P, nch, w))
                    bmb = bm.unsqueeze(1).broadcast_to((P, nch, w))
                    nc.vector.copy_predicated(dst[:, c0:c1, MAIN], amb,
                                              src[:, c0:c1, asrc])
                    nc.vector.copy_predicated(dst[:, c0:c1, MAIN], bmb,
                                              src[:, c0:c1, bsrc])
                    if not p2:
                        # refresh halos for the pass-2 chain/CPs; the mask
                        # chunk first so mask_pass(p2) unblocks early.
                        nc.scalar.copy(dst[:, c0:c1, 0:1],
                                       dst[:, c0:c1, w:w + 1])
                        nc.scalar.copy(dst[:, c0:c1, w + 1:w + 2],
                                       dst[:, c0:c1, 1:2])

            def blend_pool(i, p2):
                """Exact in-place XOR pair-swap of the TP channels.
                pass1 pairs (j-1, j) via a1[j]; pass2 pairs (j, j+1) via
                a2[j]."""
                s = st[i]
                TP = s['TP']
                which = 2 if p2 else 1
                A32 = s[f'A32_{which}']
                ti = TP[:, :, :].bitcast(I32)
                X = pxp.tile([P, NPOOL, WH], I32, tag="px", name=f"X{which}")
                if not p2:
                    # f[j] = a1[j] & (x[j] ^ x[j-1]), j in [1..w+1]
                    fs, fo = slice(1, w + 2), slice(0, w + 1)
                    osl = RIGHT
                else:
                    # f[j] = a2[j] & (x[j] ^ x[j+1]), j in [0..w]
                    fs, fo = slice(0, w + 1), slice(1, w + 2)
                    osl = LEFT
                a32 = A32[:, fs].unsqueeze(1).broadcast_to((P, NPOOL, w + 1))
                nc.gpsimd.tensor_tensor(X[:, :, fs], ti[:, :, fs],
                                        ti[:, :, fo], A.bitwise_xor)
                nc.gpsimd.tensor_tensor(X[:, :, fs], X[:, :, fs], a32,
                                        A.bitwise_and)
                nc.gpsimd.tensor_tensor(ti[:, :, MAIN], ti[:, :, MAIN],
                                        X[:, :, MAIN], A.bitwise_xor)
                nc.gpsimd.tensor_tensor(ti[:, :, MAIN], ti[:, :, MAIN],
                                        X[:, :, osl], A.bitwise_xor)
                if not p2:
                    # refresh halos of the pool channels for pass 2
                    nc.scalar.copy(TP[:, :, 0:1], TP[:, :, w:w + 1])
                    nc.scalar.copy(TP[:, :, w + 1:w + 2], TP[:, :, 1:2])

            def fixup_store(i):
                b, t = iters[i]
                hs = slice(t * P, (t + 1) * P)
                s = st[i]
                TD, TP = s['TD'], s['TP']
                NF = mk.tile([P, w], F32, tag="mk", name="NF")
                FLI = mk.tile([P, w], F32, tag="mk", name="FLI")
                FLI8 = amp.tile([P, w], I8, tag="fli8", name=f"FLI8_{i}")
                # nfm = 2*b1 - 2*b2 = 2*(a1[j+1] - a2[j-1])
                nc.vector.tensor_tensor(NF[:], s['A1f'][:, RIGHT],
                                        s['A2f'][:, LEFT], A.subtract)
                nc.vector.tensor_scalar(NF[:], NF[:], 2.0, None, A.mult)
                membership(TD[:, 0, MAIN], FLI)
                nc.vector.tensor_copy(FLI8[:], FLI[:])
                nc.vector.copy_predicated(TD[:, 3, MAIN], FLI8[:], NF[:])
                for d0, d1, t0 in TD_RUNS:
                    nc.scalar.dma_start(
                        out[b, d0:d1, hs, :].rearrange("c p w -> p c w"),
                        TD[:, t0:t0 + (d1 - d0), MAIN])
                for d0, d1, t0 in TP_RUNS:
                    nc.scalar.dma_start(
                        out[b, d0:d1, hs, :].rearrange("c p w -> p c w"),
                        TP[:, t0:t0 + (d1 - d0), MAIN])

            # ---- software-pipelined emission -------------------------------
            loads(0)
            for i in range(n):
                base1(i)
                mask_pass(i, False)
                blend_pool(i, False)
                blend_cps(i, False)
                if i + 1 < n:
                    loads(i + 1)
                mask_pass(i, True)
                base2(i)
                blend_pool(i, True)
                blend_cps(i, True)
                fixup_store(i)

    nc.compile()
    _nc_cache[key] = nc
    return nc


def kernel(world, rand_movement, rand_interact, rand_element):
    del rand_interact, rand_element
    nc = build_kernel()
    in_maps = []
    for k in range(N_CORES):
        bs = slice(k * BPC, (k + 1) * BPC)
        in_maps.append({
            "world": np.ascontiguousarray(world[bs]),
            "rand": np.ascontiguousarray(rand_movement[bs, 0]),
        })
    res = run_bass_kernel_spmd(nc, in_maps, list(range(N_CORES)))
    return np.concatenate([res.results[k]["out"] for k in range(N_CORES)], axis=0)
